# revision 3
# baseline (speedup 1.0000x reference)
"""KV-cache append kernel for Trainium2 (8 NeuronCores, batch-parallel).

Problem: nn_KvCache — given caches keys/values (B, L, H, D), per-batch
lengths, and new_keys/new_values (B, T, H, D) with per-batch new_lengths,
write the first new_lengths[b] new tokens at positions
[lengths[b], lengths[b]+new_lengths[b]) of batch b's cache and return the
full updated caches plus lengths + new_lengths.

Sharding: pure data parallel over the batch axis — core b owns batch b.
Each core does two 16 MB DRAM->DRAM bulk copies (cache passthrough) and an
indirect-DMA row scatter of up to T=128 new tokens (rows beyond
new_lengths[b] carry an out-of-bounds index and are skipped by the DMA
bounds check), plus a 1-element int add for the updated length.
"""

import numpy as np

_B, _L, _H, _D, _T = 8, 4096, 8, 128, 128
_HD = _H * _D  # 1024 floats = 4 KiB per token row
_NCORES = 8
_OOB_IDX = 1 << 20  # any index > L-1 is skipped by the scatter bounds check

_PROGRAM = None


def _get_program():
    global _PROGRAM
    if _PROGRAM is not None:
        return _PROGRAM

    import concourse.bacc as bacc
    import concourse.bass as bass
    import concourse.mybir as mybir
    import concourse.tile as tile

    f32 = mybir.dt.float32
    i32 = mybir.dt.int32

    nc = bacc.Bacc("TRN2", target_bir_lowering=False, debug=False, num_devices=_NCORES)

    k_in = nc.dram_tensor("k_in", [_L, _HD], f32, kind="ExternalInput")
    v_in = nc.dram_tensor("v_in", [_L, _HD], f32, kind="ExternalInput")
    nk_in = nc.dram_tensor("nk_in", [_T, _HD], f32, kind="ExternalInput")
    nv_in = nc.dram_tensor("nv_in", [_T, _HD], f32, kind="ExternalInput")
    idx_in = nc.dram_tensor("idx_in", [_T, 1], i32, kind="ExternalInput")
    len_in = nc.dram_tensor("len_in", [1, 2], i32, kind="ExternalInput")

    k_out = nc.dram_tensor("k_out", [_L, _HD], f32, kind="ExternalOutput")
    v_out = nc.dram_tensor("v_out", [_L, _HD], f32, kind="ExternalOutput")
    len_out = nc.dram_tensor("len_out", [1, 1], i32, kind="ExternalOutput")

    chunk = 512  # rows per bulk DMA: 512 * 4 KiB = 2 MiB

    with tile.TileContext(nc) as tc:
        with tc.tile_pool(name="sbuf", bufs=1) as pool:
            nk_t = pool.tile([_T, _HD], f32)
            nv_t = pool.tile([_T, _HD], f32)
            idx_t = pool.tile([_T, 1], i32)
            len_t = pool.tile([1, 2], i32)
            len_o = pool.tile([1, 1], i32)

            nc.sync.dma_start(out=nk_t[:], in_=nk_in[:])
            nc.sync.dma_start(out=nv_t[:], in_=nv_in[:])
            nc.sync.dma_start(out=idx_t[:], in_=idx_in[:])
            nc.sync.dma_start(out=len_t[:], in_=len_in[:])

            # Bulk cache passthrough, DRAM->DRAM, split across both HWDGE rings.
            for r in range(0, _L, chunk):
                nc.sync.dma_start(out=k_out[r : r + chunk, :], in_=k_in[r : r + chunk, :])
                nc.scalar.dma_start(out=v_out[r : r + chunk, :], in_=v_in[r : r + chunk, :])

            # Scatter the new tokens over the bulk copy. Rows t >= new_length
            # carry an OOB index and are silently skipped.
            nc.gpsimd.indirect_dma_start(
                out=k_out[:],
                out_offset=bass.IndirectOffsetOnAxis(ap=idx_t[:, :1], axis=0),
                in_=nk_t[:],
                in_offset=None,
                bounds_check=_L - 1,
                oob_is_err=False,
            )
            nc.gpsimd.indirect_dma_start(
                out=v_out[:],
                out_offset=bass.IndirectOffsetOnAxis(ap=idx_t[:, :1], axis=0),
                in_=nv_t[:],
                in_offset=None,
                bounds_check=_L - 1,
                oob_is_err=False,
            )

            nc.vector.tensor_add(out=len_o[:, :], in0=len_t[:, 0:1], in1=len_t[:, 1:2])
            nc.sync.dma_start(out=len_out[:], in_=len_o[:])

    nc.compile()
    _PROGRAM = nc
    return nc


def _build_in_maps(keys, values, lengths, new_keys, new_values, new_lengths):
    in_maps = []
    for b in range(_B):
        l = int(lengths[b])
        nl = int(new_lengths[b])
        idx = (np.arange(_T, dtype=np.int32) + l).reshape(_T, 1)
        if nl < _T:
            idx[nl:] = _OOB_IDX
        in_maps.append(
            {
                "k_in": np.ascontiguousarray(keys[b]).reshape(_L, _HD),
                "v_in": np.ascontiguousarray(values[b]).reshape(_L, _HD),
                "nk_in": np.ascontiguousarray(new_keys[b]).reshape(_T, _HD),
                "nv_in": np.ascontiguousarray(new_values[b]).reshape(_T, _HD),
                "idx_in": idx,
                "len_in": np.array([[l, nl]], dtype=np.int32),
            }
        )
    return in_maps


def _run(keys, values, lengths, new_keys, new_values, new_lengths, **spmd_kwargs):
    from concourse.bass_utils import run_bass_kernel_spmd

    nc = _get_program()
    in_maps = _build_in_maps(keys, values, lengths, new_keys, new_values, new_lengths)
    out = run_bass_kernel_spmd(nc, in_maps, core_ids=list(range(_NCORES)), **spmd_kwargs)

    res = out.results
    upd_keys = np.stack([res[b]["k_out"].reshape(_L, _H, _D) for b in range(_B)])
    upd_values = np.stack([res[b]["v_out"].reshape(_L, _H, _D) for b in range(_B)])
    upd_lengths = np.array([res[b]["len_out"][0, 0] for b in range(_B)], dtype=np.int32)
    return (upd_keys, upd_values, upd_lengths), out


def kernel(keys, values, lengths, new_keys, new_values, new_lengths):
    keys = np.asarray(keys, dtype=np.float32)
    values = np.asarray(values, dtype=np.float32)
    lengths = np.asarray(lengths, dtype=np.int32)
    new_keys = np.asarray(new_keys, dtype=np.float32)
    new_values = np.asarray(new_values, dtype=np.float32)
    new_lengths = np.asarray(new_lengths, dtype=np.int32)

    outputs, _ = _run(keys, values, lengths, new_keys, new_values, new_lengths)
    return outputs
